# revision 37
# baseline (speedup 1.0000x reference)
"""DGCNN (3x DynamicEdgeConv + MLP head) Trainium2 Bass kernel.

Data-parallel over the batch axis: 8 graphs -> 8 NeuronCores, weights
replicated. Per graph (N=2048 nodes, K=16 neighbors):
  - kNN scores s'[i,j] = 2<x_i,x_j> - |x_j|^2 via PE matmuls (argmax == knn)
  - exact per-row top-16 via DVE max8 / max_index / match_replace (2 rounds)
  - neighbor gather with gpsimd ap_gather on [C, N] feature layout
  - edge MLP: first linear accumulated on PE from gathered x_j and
    broadcast x_i terms, relu(+bias) on ACT, second linear on PE,
    max-pool over k via DVE segmented reduce
  - final MLP in [C, N] layout, classifier + log_softmax in node-major
"""

import numpy as np

import concourse.bacc as bacc
import concourse.mybir as mybir
from concourse import masks, tile
from concourse import bass_utils

F32 = mybir.dt.float32
AX = mybir.AxisListType
OP = mybir.AluOpType
ACTF = mybir.ActivationFunctionType

B, N, K = 8, 2048, 16
DBG_SKIP = set()
NT = N // 128  # node tiles per graph
NEG = -3.0e38

WEIGHT_NAMES = [
    "w1a_w", "w1a_b", "w1b_w", "w1b_b",
    "w2a_w", "w2a_b", "w2b_w", "w2b_b",
    "w3a_w", "w3a_b", "w3b_w", "w3b_b",
    "m1_w", "m1_b", "m2_w", "m2_b", "m3_w", "m3_b", "m4_w", "m4_b",
]
WEIGHT_SHAPES = {
    "w1a_w": (2, 64), "w1a_b": (64,), "w1b_w": (64, 64), "w1b_b": (64,),
    "w2a_w": (128, 128), "w2a_b": (128,), "w2b_w": (128, 128), "w2b_b": (128,),
    "w3a_w": (256, 256), "w3a_b": (256,), "w3b_w": (256, 256), "w3b_b": (256,),
    "m1_w": (448, 512), "m1_b": (512,), "m2_w": (512, 512), "m2_b": (512,),
    "m3_w": (512, 256), "m3_b": (256,), "m4_w": (256, 2), "m4_b": (2,),
}


def ceil_div(a, b):
    return (a + b - 1) // b


def _chunks(n, c=128):
    """[(start, size), ...] splitting n into chunks of <=c."""
    return [(s, min(c, n - s)) for s in range(0, n, c)]


def build_nc(repeat_loop=False, dbg_stop=None):
    nc = bacc.Bacc("TRN2", target_bir_lowering=False, debug=False)

    x_d = nc.dram_tensor("x", [N, 1], F32, kind="ExternalInput")
    w_d = {
        n: nc.dram_tensor(n, list(WEIGHT_SHAPES[n]), F32, kind="ExternalInput")
        for n in WEIGHT_NAMES
    }
    steps_d = None
    if repeat_loop:
        steps_d = nc.dram_tensor("steps", [1, 1], mybir.dt.uint32,
                                 kind="ExternalInput")
    out_d = nc.dram_tensor("out", [N, 2], F32, kind="ExternalOutput")

    with tile.TileContext(nc) as tc:
        _emit(nc, tc, x_d, w_d, out_d, steps_d, dbg_stop)
    nc.compile()
    return nc


def _emit(nc, tc, x_d, w_d, out_d, steps_d, dbg_stop=None):
    from contextlib import ExitStack

    ctx = ExitStack()
    with ctx:
        const = ctx.enter_context(tc.tile_pool(name="const", bufs=1))
        wpool = ctx.enter_context(tc.tile_pool(name="wpool", bufs=1))
        feats = ctx.enter_context(tc.tile_pool(name="feats", bufs=1))

        # ---- constants ----
        ident = const.tile([128, 128], F32)
        masks.make_identity(nc, ident[:])
        rep16 = const.tile([16, 128], F32)
        nc.vector.tensor_copy(
            rep16[:].rearrange("p (r c) -> p r c", r=8),
            ident[:16, :16].unsqueeze(1).to_broadcast([16, 8, 16]))
        ones1 = const.tile([1, N], F32)
        nc.vector.memset(ones1[:], 1.0)
        ones_col = const.tile([128, 1], F32)
        nc.vector.memset(ones_col[:], 1.0)

        # ---- load weights ----
        def load2d(name, row_splits=None):
            rows, cols = WEIGHT_SHAPES[name]
            ts = []
            splits = row_splits or _chunks(rows)
            for i, (s, p) in enumerate(splits):
                t = wpool.tile([p, cols], F32, tag=f"w_{name}_{i}",
                               name=f"w_{name}_{i}")
                nc.sync.dma_start(t[:], w_d[name].ap()[s:s + p, :])
                ts.append(t)
            return ts

        def load_bias_col(name):
            (n_,) = WEIGHT_SHAPES[name]
            ts = []
            for i, (s, p) in enumerate(_chunks(n_)):
                t = wpool.tile([p, 1], F32, tag=f"b_{name}_{i}",
                               name=f"b_{name}_{i}")
                nc.sync.dma_start(t[:], w_d[name].ap()[s:s + p].unsqueeze(-1))
                ts.append(t)
            return ts

        w1a = load2d("w1a_w")[0]   # [2, 64]
        w1b = load2d("w1b_w")[0]   # [64, 64]
        w2a = load2d("w2a_w")[0]   # [128, 128]
        w2b = load2d("w2b_w")[0]   # [128, 128]
        w3a = load2d("w3a_w")      # 2 x [128, 256]
        w3b = load2d("w3b_w")      # 2 x [128, 256]
        # row splits aligned with feature chunks [64, 128, 128, 128]
        m1 = load2d("m1_w", row_splits=[(0, 64), (64, 128), (192, 128), (320, 128)])
        m2 = load2d("m2_w")        # 4 x [128, 512]
        m3 = load2d("m3_w")        # 4 x [128, 256]
        m4 = load2d("m4_w")        # 2 x [128, 2]
        b1a = load_bias_col("w1a_b")[0]   # [64,1]
        b1b = load_bias_col("w1b_b")[0]
        b2a = load_bias_col("w2a_b")[0]   # [128,1]
        b2b = load_bias_col("w2b_b")[0]
        b3a = load_bias_col("w3a_b")      # 2 x [128,1]
        b3b = load_bias_col("w3b_b")
        bm1 = load_bias_col("m1_b")       # 4 x [128,1]
        bm2 = load_bias_col("m2_b")
        bm3 = load_bias_col("m3_b")
        bm4 = wpool.tile([1, 2], F32)
        nc.sync.dma_start(bm4[:], w_d["m4_b"].ap().unsqueeze(0))

        # wdiff = Wx - Wd  (first C_in rows minus last C_in rows of w*a)
        # Wd halves need their own partition-0 tiles (matmul lhsT base
        # partition must match rhs), so load them from DRAM directly.
        wd1 = wpool.tile([1, 64], F32)
        nc.sync.dma_start(wd1[:], w_d["w1a_w"].ap()[1:2, :])
        wdiff1 = wpool.tile([1, 64], F32)
        nc.vector.tensor_tensor(wdiff1[:], w1a[0:1, :], wd1[:], op=OP.subtract)
        wd2 = wpool.tile([64, 128], F32)
        nc.sync.dma_start(wd2[:], w_d["w2a_w"].ap()[64:128, :])
        wdiff2 = wpool.tile([64, 128], F32)
        nc.vector.tensor_tensor(wdiff2[:], w2a[0:64, :], wd2[:], op=OP.subtract)
        # layer 3: Wx = w3a rows 0..127 (tile 0), Wd = rows 128..255 (tile 1)
        wd3 = w3a[1]                           # [128, 256]
        wdiff3 = wpool.tile([128, 256], F32)
        nc.vector.tensor_tensor(wdiff3[:], w3a[0][:], w3a[1][:], op=OP.subtract)
        # bf16 lo/hi halves of Wd3 for the packed-bf16 L3 gather path
        BF16 = mybir.dt.bfloat16
        wd3hi_f = wpool.tile([64, 256], F32)
        nc.sync.dma_start(wd3hi_f[:], w_d["w3a_w"].ap()[192:256, :])
        wd3lo_b = wpool.tile([64, 256], BF16)
        nc.scalar.activation(wd3lo_b[:], w3a[1][0:64, :], ACTF.Copy)
        wd3hi_b = wpool.tile([64, 256], BF16)
        nc.scalar.activation(wd3hi_b[:], wd3hi_f[:], ACTF.Copy)

        # ---- feature tensors ----
        x0T = feats.tile([1, N], F32)
        nc.sync.dma_start(x0T[:], x_d.ap().rearrange("n 1 -> 1 n"))
        x1T = feats.tile([128, N], F32)  # rows 64-127: replica for paired gather
        x2T = feats.tile([128, N], F32)
        x3Ta = feats.tile([128, N], F32)
        x3Tb = feats.tile([128, N], F32)

        body_ctx = ExitStack()
        with body_ctx:
            scr = body_ctx.enter_context(tc.tile_pool(name="scr", bufs=2))
            pss = body_ctx.enter_context(
                tc.tile_pool(name="pss", bufs=1, space="PSUM"))
            psi = body_ctx.enter_context(
                tc.tile_pool(name="psi", bufs=2, space="PSUM"))
            pse = body_ctx.enter_context(
                tc.tile_pool(name="pse", bufs=1, space="PSUM"))

            if steps_d is not None:
                steps_sb = const.tile([1, 1], mybir.dt.uint32)
                nc.sync.dma_start(steps_sb[:], steps_d.ap())
                (_, (steps_val,)) = nc.values_load_multi_w_load_instructions(
                    steps_sb[:], min_val=1, max_val=1000000,
                    skip_runtime_bounds_check=True)
                body_ctx.enter_context(tc.For_i(0, steps_val, 1))

            def edge_layer(lname, feat_in, c_in, c_out, wd_lhs, wdiff_lhs,
                           ba_col, wb, bb_col, feat_out, factored_c=None,
                           pair_src=None, packed_lhs=None):
                """One DynamicEdgeConv layer.

                feat_in: list of [p, N] APs covering c_in rows (or 1 for L1)
                wd_lhs / wdiff_lhs: [kc][m] lhsT tiles
                feat_out: list of [p, N] APs covering c_out
                factored_c: L1 -- gather precomputed cT instead of feat_in
                pair_src: [128, N] tile holding the gather source replicated
                          in both partition halves -> one gather per 2 tiles
                """
                kc_in = _chunks(c_in)
                mo = _chunks(c_out)

                # --- layer-level: 2*xT chunks and -sq row ---
                twox = []
                for ci, (s, p) in enumerate(kc_in):
                    t = scr.tile([128, N], F32, tag=f"twox{ci}", bufs=1)
                    nc.vector.tensor_scalar_mul(t[:p, :], feat_in[ci][:, :], 2.0)
                    twox.append(t)
                xsq = scr.tile([128, N], F32, tag="xsq", bufs=1)
                negsq = scr.tile([1, N], F32, tag="negsq", bufs=1)
                for j in range(4):
                    fsl = slice(j * 512, (j + 1) * 512)
                    sq_ps = psi.tile([1, 512], F32, tag="tmp")
                    for ci, (s, p) in enumerate(kc_in):
                        nc.vector.tensor_tensor(
                            xsq[:p, fsl], feat_in[ci][:, fsl], feat_in[ci][:, fsl],
                            op=OP.mult)
                        nc.tensor.matmul(
                            sq_ps[:], ones_col[:p, :], xsq[:p, fsl],
                            start=(ci == 0), stop=(ci == len(kc_in) - 1))
                    nc.scalar.activation(
                        negsq[:, fsl], sq_ps[:], ACTF.Copy, scale=-1.0)

                def tile_head(t):
                    """s'-matmul + top-16 for node tile t -> iu [128,16] u16."""
                    ts0 = t * 128
                    tsl = slice(ts0, ts0 + 128)
                    s_ps = pss.tile([128, N], F32, tag="s_ps", name="s_ps")
                    for j in range(4):
                        fsl = slice(j * 512, (j + 1) * 512)
                        for ci, (s, p) in enumerate(kc_in):
                            nc.tensor.matmul(
                                s_ps[:, fsl], feat_in[ci][:, tsl],
                                twox[ci][:p, fsl], start=(ci == 0), stop=False)
                        nc.tensor.matmul(
                            s_ps[:, fsl], ones1[:, tsl], negsq[:, fsl],
                            start=False, stop=True)
                    s_sb = scr.tile([128, N], F32, tag="s_sb", name="s_sb")
                    for j in range(2):
                        fsl = slice(j * 1024, (j + 1) * 1024)
                        nc.scalar.activation(s_sb[:, fsl], s_ps[:, fsl], ACTF.Copy)
                    v = scr.tile([128, 16], F32, tag="v16", name="v16")
                    iu = scr.tile([128, 16], mybir.dt.uint16, tag="iu16",
                                  name="iu16")
                    nc.vector.max(out=v[:, 0:8], in_=s_sb[:])
                    nc.vector.max_index(out=iu[:, 0:8], in_max=v[:, 0:8],
                                        in_values=s_sb[:])
                    nc.vector.match_replace(out=s_sb[:], in_to_replace=v[:, 0:8],
                                            in_values=s_sb[:], imm_value=NEG)
                    nc.vector.max(out=v[:, 8:16], in_=s_sb[:])
                    nc.vector.max_index(out=iu[:, 8:16], in_max=v[:, 8:16],
                                        in_values=s_sb[:])
                    return iu

                def tile_tail(t, xgs):
                    """Edge MLP + pool for node tile t. xgs: [p, N] APs."""
                    ts0 = t * 128
                    tsl = slice(ts0, ts0 + 128)
                    if factored_c is not None:
                        hr = xgs[0]
                        nc.vector.tensor_tensor(
                            hr, hr,
                            aT1[:, tsl].unsqueeze(-1).to_broadcast(
                                [c_out, 128, K]),
                            op=OP.add)
                        nc.scalar.activation(hr, hr, ACTF.Relu,
                                             bias=ba_col[0][:])
                        for j in range(4):
                            fsl = slice(j * 512, (j + 1) * 512)
                            h2_ps = pse.tile([c_out, 512], F32, tag="h2",
                                             name="h2_ps")
                            nc.tensor.matmul(h2_ps[:], wb[:], hr[:, fsl])
                            nc.vector.tensor_reduce(
                                out=feat_out[0][:, ts0 + j * 32:
                                                ts0 + (j + 1) * 32],
                                in_=h2_ps[:].rearrange("c (n k) -> c n k", k=K),
                                axis=AX.X, op=OP.max)
                        nc.vector.tensor_scalar(
                            feat_out[0][:, tsl], feat_out[0][:, tsl],
                            bb_col[0][:], None, op0=OP.add)
                    else:
                        for j in range(4):
                            fsl = slice(j * 512, (j + 1) * 512)
                            nsl = slice(ts0 + j * 32, ts0 + (j + 1) * 32)
                            h1cs = []
                            for mi, (ms, mp) in enumerate(mo):
                                pre_ps = pse.tile([128, 512], F32, tag="pre",
                                                  name="pre_ps")
                                if packed_lhs is not None:
                                    xgv = xgs[0].bitcast(
                                        mybir.dt.bfloat16).rearrange(
                                        "c (n t) -> c n t", t=2)
                                    nc.tensor.matmul(
                                        pre_ps[:mp, :],
                                        packed_lhs[0][ms:ms + mp]
                                        if False else
                                        packed_lhs[0][:, ms:ms + mp],
                                        xgv[:, fsl, 0:1],
                                        start=True, stop=False)
                                    nc.tensor.matmul(
                                        pre_ps[:mp, :],
                                        packed_lhs[1][:, ms:ms + mp],
                                        xgv[:, fsl, 1:2],
                                        start=False, stop=False)
                                else:
                                    for ci, (s, p) in enumerate(kc_in):
                                        nc.tensor.matmul(
                                            pre_ps[:mp, :], wd_lhs[ci][mi][:],
                                            xgs[ci][:, fsl],
                                            start=(ci == 0), stop=False)
                                for ci, (s, p) in enumerate(kc_in):
                                    nc.tensor.matmul(
                                        pre_ps[:mp, :], wdiff_lhs[ci][mi][:],
                                        feat_in[ci][:, nsl].unsqueeze(-1)
                                        .to_broadcast([p, 32, K]),
                                        start=False,
                                        stop=(ci == len(kc_in) - 1))
                                h1c = scr.tile([128, 512], F32, tag=f"h1_{mi}",
                                               name="h1c")
                                nc.scalar.activation(
                                    h1c[:mp, :], pre_ps[:mp, :], ACTF.Relu,
                                    bias=ba_col[mi][:])
                                h1cs.append(h1c)
                            for gi, (gs, gp2) in enumerate(mo):
                                h2_ps = pse.tile([128, 512], F32, tag="h2",
                                                 name="h2_ps")
                                for mi, (ms, mp) in enumerate(mo):
                                    nc.tensor.matmul(
                                        h2_ps[:gp2, :],
                                        wb[mi][:, gs:gs + gp2],
                                        h1cs[mi][:mp, :],
                                        start=(mi == 0),
                                        stop=(mi == len(mo) - 1))
                                nc.vector.tensor_reduce(
                                    out=feat_out[gi][:, nsl],
                                    in_=h2_ps[:gp2, :].rearrange(
                                        "c (n k) -> c n k", k=K),
                                    axis=AX.X, op=OP.max)
                        for gi, (gs, gp2) in enumerate(mo):
                            nc.vector.tensor_scalar(
                                feat_out[gi][:, tsl], feat_out[gi][:, tsl],
                                bb_col[gi][:], None, op0=OP.add)

                gp = c_out if factored_c is not None else c_in
                if packed_lhs is not None:
                    gp = 64  # bf16 channel pairs packed into fp32 slots
                if pair_src is not None:
                    assert gp <= 64
                    for tp in range(NT // 2):
                        ta, tb = 2 * tp, 2 * tp + 1
                        iua = tile_head(ta)
                        iub = tile_head(tb)
                        iu_rep = scr.tile([128, 128], mybir.dt.uint16,
                                          tag="iu_rep", name="iu_rep")
                        nc.vector.tensor_copy(
                            iu_rep[:, 0:64].rearrange("p (r k) -> p r k", r=4),
                            iua[:].unsqueeze(1).to_broadcast([128, 4, 16]))
                        nc.vector.tensor_copy(
                            iu_rep[:, 64:128].rearrange("p (r k) -> p r k", r=4),
                            iub[:].unsqueeze(1).to_broadcast([128, 4, 16]))
                        idx16 = scr.tile([128, 128], mybir.dt.uint16,
                                         tag="idx16", name="idx16")
                        nc.sync.dma_start_transpose(idx16[:], iu_rep[:])
                        xg = scr.tile([128, N], F32, tag="xg0", name="xg0")
                        nc.gpsimd.ap_gather(
                            out_ap=xg[:], in_ap=pair_src[:],
                            idxs_ap=idx16[:].bitcast(mybir.dt.int16),
                            channels=128, num_elems=N, d=1, num_idxs=N)
                        xgb = scr.tile([64, N], F32, tag="xg1", name="xg1")
                        nc.sync.dma_start(xgb[:], xg[64:128, :])
                        tile_tail(ta, [xg[:gp, :]])
                        tile_tail(tb, [xgb[:gp, :]])
                else:
                    for t in range(NT):
                        iu = tile_head(t)
                        iu_rep = scr.tile([128, 128], mybir.dt.uint16,
                                          tag="iu_rep", name="iu_rep")
                        nc.vector.tensor_copy(
                            iu_rep[:].rearrange("p (r k) -> p r k", r=8),
                            iu[:].unsqueeze(1).to_broadcast([128, 8, 16]))
                        idx16 = scr.tile([128, 128], mybir.dt.uint16,
                                         tag="idx16", name="idx16")
                        nc.sync.dma_start_transpose(idx16[:], iu_rep[:])
                        srcs = ([factored_c] if factored_c is not None
                                else feat_in)
                        xgs = []
                        for ci, srct in enumerate(srcs):
                            p = srct.shape[0] if hasattr(srct, "shape") else gp
                            xg = scr.tile([128, N], F32, tag=f"xg{ci}",
                                          name="xg")
                            nc.gpsimd.ap_gather(
                                out_ap=xg[:p, :], in_ap=srct[:, :],
                                idxs_ap=idx16[:p, :].bitcast(mybir.dt.int16),
                                channels=p, num_elems=N, d=1, num_idxs=N)
                            xgs.append(xg[:p, :])
                        tile_tail(t, xgs)

            # ---------- layer 1 (factored: gather cT [64, N]) ----------
            # aT1 = wdiff1.T @ x0T ; cT1 = wd1.T @ x0T (replicated halves)
            aT1 = scr.tile([64, N], F32, tag="aT1", bufs=1)
            cT1 = scr.tile([128, N], F32, tag="cT1", bufs=1)
            for j in range(4):
                fsl = slice(j * 512, (j + 1) * 512)
                a_ps = psi.tile([64, 512], F32, tag="tmp")
                nc.tensor.matmul(a_ps[:], wdiff1[:], x0T[:, fsl])
                nc.scalar.activation(aT1[:, fsl], a_ps[:], ACTF.Copy)
                c_ps = psi.tile([64, 512], F32, tag="tmp")
                nc.tensor.matmul(c_ps[:], wd1[:], x0T[:, fsl])
                nc.scalar.activation(cT1[0:64, fsl], c_ps[:], ACTF.Copy)
            nc.sync.dma_start(cT1[64:128, :], cT1[0:64, :])

            edge_layer("l1", [x0T], 1, 64, None, None, [b1a], w1b, [b1b],
                       [x1T[0:64, :]], factored_c=cT1[0:64, :], pair_src=cT1)

            if dbg_stop == 1:
                nc.sync.dma_start(out_d.ap().rearrange("n c -> c n"), x1T[0:2, :])
                return

            # ---------- layer 2 ----------
            wd2_lhs = [[wd2]]          # kc=0 -> m chunks
            wdiff2_lhs = [[wdiff2]]
            nc.sync.dma_start(x1T[64:128, :], x1T[0:64, :])
            edge_layer("l2", [x1T[0:64, :]], 64, 128, wd2_lhs, wdiff2_lhs,
                       [b2a], [w2b], [b2b], [x2T], pair_src=x1T)

            if dbg_stop == 2:
                nc.sync.dma_start(out_d.ap().rearrange("n c -> c n"), x2T[0:2, :])
                return

            # ---------- layer 3 (packed-bf16 paired gather) ----------
            # x2pk[r, e] packs bf16(x2[r, e]) | bf16(x2[r+64, e]) in one f32
            x2hi_f = scr.tile([64, N], F32, tag="xsq", bufs=1, name="x2hi_f")
            nc.sync.dma_start(x2hi_f[:], x2T[64:128, :])
            x2pk = scr.tile([128, N], F32, tag="x2pk", bufs=1)
            pk3 = x2pk[0:64, :].bitcast(mybir.dt.bfloat16).rearrange(
                "c (n t) -> c n t", t=2)
            nc.scalar.activation(pk3[:, :, 0:1],
                                 x2T[0:64, :].unsqueeze(-1), ACTF.Copy)
            nc.scalar.activation(pk3[:, :, 1:2],
                                 x2hi_f[:].unsqueeze(-1), ACTF.Copy)
            nc.sync.dma_start(x2pk[64:128, :], x2pk[0:64, :])

            wdiff3_lhs = [[wdiff3[:, 0:128], wdiff3[:, 128:256]]]
            edge_layer("l3", [x2T], 128, 256, None, wdiff3_lhs, b3a,
                       w3b, b3b, [x3Ta, x3Tb], pair_src=x2pk,
                       packed_lhs=(wd3lo_b, wd3hi_b))

            if dbg_stop == 3:
                nc.sync.dma_start(out_d.ap().rearrange("n c -> c n"), x3Ta[0:2, :])
                return

            # ---------- final MLP (n-chunked: 512 nodes end-to-end) ----------
            featc = [x1T[0:64, :], x2T, x3Ta, x3Tb]

            for j in range(4):
                fsl = slice(j * 512, (j + 1) * 512)
                h1c = [scr.tile([128, 512], F32, tag=f"mh1_{m}", bufs=1,
                                name=f"mh1_{m}") for m in range(4)]
                for m in range(4):
                    ps = pse.tile([128, 512], F32, tag="pre")
                    for ci, wc in enumerate(m1):
                        nc.tensor.matmul(
                            ps[:], wc[:, m * 128:(m + 1) * 128],
                            featc[ci][:, fsl],
                            start=(ci == 0), stop=(ci == 3))
                    nc.scalar.activation(h1c[m][:], ps[:], ACTF.Relu,
                                         bias=bm1[m][:])
                h2c = [scr.tile([128, 512], F32, tag=f"mh2_{m}", bufs=1,
                                name=f"mh2_{m}") for m in range(4)]
                for m in range(4):
                    ps = pse.tile([128, 512], F32, tag="pre")
                    for ci in range(4):
                        nc.tensor.matmul(
                            ps[:], m2[ci][:, m * 128:(m + 1) * 128], h1c[ci][:],
                            start=(ci == 0), stop=(ci == 3))
                    nc.scalar.activation(h2c[m][:], ps[:], ACTF.Relu,
                                         bias=bm2[m][:])
                h3c = [scr.tile([128, 512], F32, tag=f"mh3_{m}", bufs=1,
                                name=f"mh3_{m}") for m in range(2)]
                for m in range(2):
                    ps = pse.tile([128, 512], F32, tag="pre")
                    for ci in range(4):
                        nc.tensor.matmul(
                            ps[:], m3[ci][:, m * 128:(m + 1) * 128], h2c[ci][:],
                            start=(ci == 0), stop=(ci == 3))
                    nc.scalar.activation(h3c[m][:], ps[:], ACTF.Relu,
                                         bias=bm3[m][:])

                # classifier + log_softmax per 128-node subtile
                for st in range(4):
                    t0 = j * 512 + st * 128
                    tsl = slice(t0, t0 + 128)
                    lsl = slice(st * 128, (st + 1) * 128)
                    o_ps = psi.tile([128, 2], F32, tag="tmp")
                    nc.tensor.matmul(o_ps[:], h3c[0][:, lsl], m4[0][:],
                                     start=True, stop=False)
                    nc.tensor.matmul(o_ps[:], h3c[1][:, lsl], m4[1][:],
                                     start=False, stop=False)
                    nc.tensor.matmul(o_ps[:], ones1[:, tsl], bm4[:],
                                     start=False, stop=True)
                    mx = scr.tile([128, 1], F32, tag="mx")
                    nc.vector.tensor_reduce(out=mx[:], in_=o_ps[:], axis=AX.X,
                                            op=OP.max)
                    hm = scr.tile([128, 2], F32, tag="hm")
                    nc.vector.tensor_scalar(hm[:], o_ps[:], mx[:], None,
                                            op0=OP.subtract)
                    ex = scr.tile([128, 2], F32, tag="ex")
                    ssum = scr.tile([128, 1], F32, tag="ssum")
                    nc.scalar.activation(ex[:], hm[:], ACTF.Exp,
                                         accum_out=ssum[:])
                    lns = scr.tile([128, 1], F32, tag="lns")
                    nc.scalar.activation(lns[:], ssum[:], ACTF.Ln)
                    res = scr.tile([128, 2], F32, tag="res")
                    nc.vector.tensor_scalar(res[:], hm[:], lns[:], None,
                                            op0=OP.subtract)
                    nc.sync.dma_start(out_d.ap()[tsl, :], res[:])


_NC_CACHE = {}


def _get_nc(repeat_loop=False):
    key = repeat_loop
    if key not in _NC_CACHE:
        _NC_CACHE[key] = build_nc(repeat_loop)
    return _NC_CACHE[key]


def kernel(**inputs):
    nc = _get_nc()
    in_maps = []
    for g in range(B):
        m = {"x": np.ascontiguousarray(np.asarray(inputs["x"][g], np.float32))}
        for w in WEIGHT_NAMES:
            m[w] = np.ascontiguousarray(np.asarray(inputs[w], np.float32))
        in_maps.append(m)
    res = bass_utils.run_bass_kernel_spmd(nc, in_maps, core_ids=list(range(B)))
    return np.stack([res.results[g]["out"] for g in range(B)], axis=0)
